# revision 54
# baseline (speedup 1.0000x reference)
"""Trainium2 Bass kernel for nn_CrossInferenceBlock (bilinear cross attention).

Computation (T=256, S=256, F=1024, A=256):
    theta = (x @ a_w + a_b).reshape(T, S, A)
    phi   = (x @ b_w + b_b).reshape(T, S, A)
    feats = (x @ g_w + g_b).reshape(T, S, F)
    attn  = einsum("tsa,tra->tsr", theta, phi)
    out   = einsum("tsr,trf->tsf", attn, feats) / (S + T)

Sharding: data-parallel over t — each of the 8 cores takes 32 contiguous
t-slices; the Linear weights are replicated.

The fp16 matmul stream (1792 N=512 + 128 N=256 MMs/core) is the
roofline: ~401us at the warm 2.4GHz issue rate (N/2.4 + 2.5ns NX).
Everything else is startup/tail engineering, driven by trace analysis:

  - The DMA subsystem is PACKET-RATE limited (~100 packets/us aggregate
    across queues; packet size = SBUF-side per-partition contiguous
    run). 2KB-run DMAs cap a queue near 50GB/s; 16KB runs reach the
    full ~358GB/s. Completion semaphores ride the same packet path, so
    a chunk's "ready" signal trails later chunks' data by ~1-3us.
  - Startup: group-0 x streams on GpSimd as a kt-quad (8KB runs) + two
    kt-pairs (4KB runs); gw mirrors that on Scalar. aw/bw (needed only
    ~25us in) queue BEHIND gw on Scalar so the per-queue FIFO keeps
    them out of the critical window. All small biases are host-packed
    into ONE [P, F+4] fp16 tensor (a transposed ab/bb load would emit
    256 tiny packets that clog the packet-limited phase).
  - ~150 dummy warm-up matmuls bridge engine-init (~7.5us) to first
    data (~17us): PE_HAM reaches 8/8 (2.4GHz) at ~11us and the PE is
    never idle after, so the clock gate never re-throttles (one idle
    MID window would cost ~10us).
  - Group 0 computes feats for t-PAIRS jointly, kt-outer, across all 8
    PSUM banks: 16 MMs (~3.4us) per arriving x/gw kt chunk matches the
    delivery cadence. theta/phi follow when a_w/b_w land; ps_a-pool
    psums evict first so the next stage's banks free earliest.
  - theta/phi matmuls process a PAIR of t-slices per instruction (N=512).
  - PSUM evictions alternate Vector/Scalar so eviction chains never gate
    the PE; output is stored fp16 (host upcasts), 2KB DMA rows, one
    store per (t, mt) on Sync; the final store splits across the
    Scalar/GpSimd queues to shorten the end-of-kernel tail.

Matmuls run in fp16 (fp32 PSUM accumulation). fp8 was evaluated and
rejected: TRN e4m3 gives ~3.5% rel error on this data (tolerance 2e-2)
and DoubleRow is only ~1.5x, so no precision/speed tradeoff works.
N=1024 matmuls (2-bank PSUM) are rejected by the backend ISA check.
Note: under sustained load the chip can enter P0 (PE ~2.0GHz), which
inflates measurements by ~1.2x — compare runs on a cool device.
"""

import numpy as np

import concourse.bass as bass
import concourse.bacc as bacc
import concourse.tile as tile
from concourse import mybir
from concourse.bass_utils import run_bass_kernel_spmd

T, S, F, A = 256, 256, 1024, 256
N_CORES = 8
T_LOC = T // N_CORES          # 32 t-slices per core
P = 128
KT = F // P                   # 8 contraction tiles over F
MT_A = A // P                 # 2 output tiles over A
MT_S = S // P                 # 2 tiles over s (rows of one t-slice)
NF = 512                      # matmul free-dim chunk for F-wide outputs
NC_F = F // NF                # 2 chunks
TG = 4                        # t-slices fetched per input DMA group
NG = T_LOC // TG              # 8 DMA groups per core
N_WARM = 150                  # warm-up matmuls bridging until data lands (~18.5us)
OUT_SCALE = 1.0 / (S + T)

F16 = mybir.dt.float16
F32 = mybir.dt.float32

_COMPILED = None


def _build():
    nc = bacc.Bacc("TRN2", target_bir_lowering=False, debug=False)

    # All inputs are host-prearranged so every DMA reads per-partition
    # CONTIGUOUS runs, keeping HBM transfers at full rate.
    # x: (NG, P, KT, TG, S) with t = g*TG + ti, f = kt*P + p.
    x_d = nc.dram_tensor("x", [NG, P, KT, TG, S], F16, kind="ExternalInput")
    aw_d = nc.dram_tensor("aw", [P, KT, MT_A, P], F16, kind="ExternalInput")
    bw_d = nc.dram_tensor("bw", [P, KT, MT_A, P], F16, kind="ExternalInput")
    gw_d = nc.dram_tensor("gw", [P, KT, F], F16, kind="ExternalInput")
    # biases: host-packed per-partition rows [gb_rep(F) | ab(MT_A) | bb(MT_A)]
    # fp16 — ONE 128-packet DMA instead of a broadcast + two transposed
    # tiny-packet DMAs that would clog the packet-limited startup phase.
    bias_d = nc.dram_tensor(
        "bias", [P, F + 2 * MT_A], F16, kind="ExternalInput"
    )
    out_d = nc.dram_tensor("out", [T_LOC, S, F], F16, kind="ExternalOutput")

    x_ap = x_d.ap()
    aw_ap = aw_d.ap()
    bw_ap = bw_d.ap()
    gw_ap = gw_d.ap()
    bias_ap = bias_d.ap()
    out_ap = out_d.ap()

    with tile.TileContext(nc) as tc:
        with (
            tc.tile_pool(name="const", bufs=1) as const,
            tc.tile_pool(name="xin", bufs=3) as xin,
            tc.tile_pool(name="proj", bufs=3) as proj,
            tc.tile_pool(name="fsb", bufs=6) as fsb,
            tc.tile_pool(name="asb", bufs=4) as asb,
            tc.tile_pool(name="osb", bufs=6) as osb,
            tc.tile_pool(name="ps_a", bufs=4, space="PSUM") as ps_a,
            tc.tile_pool(name="ps_b", bufs=4, space="PSUM") as ps_b,
        ):
            # --- PE_HAM warm-up: dummy matmuls on a memset tile. These
            # depend only on engine init (~6.5us), not on any DMA, so the
            # clock gate reaches 8/8 before the first real matmul.
            warm_sb = const.tile([P, P], F16)
            nc.vector.memset(warm_sb[:], 0.0)
            warm_ps = ps_a.tile([P, NF], F32, tag="ps")
            for _ in range(N_WARM):
                nc.tensor.matmul(
                    warm_ps[:, :P], lhsT=warm_sb[:], rhs=warm_sb[:],
                    start=True, stop=True,
                )

            # --- Startup DMAs. The DMA subsystem is packet-rate limited
            # (~100 packets/us aggregate; packet size = SBUF-side
            # per-partition run). Group-0 x streams as a kt-QUAD (8KB
            # runs) then two kt-pairs: the quad has the same packet count
            # as a pair (so the first-matmul gate is no later) but lands
            # 2x the work backlog, absorbing DMA jitter. gw mirrors this
            # with a half + two pairs on Scalar.
            # Four kt-PAIR chunks (4KB runs) for both streams. Measured
            # against alternatives: kt-singles gate ~1us earlier but
            # double the early packet count and starve mid-startup; a
            # coarse kt4-7 quad halves packets but its data+semaphore
            # loses the race against the f01 consumption front (2us PE
            # gap on slow-DMA runs). Pairs starve nowhere: chunk k lands
            # at ~gate+1.4k us vs the ~gate+1.7k us consumption front.
            xt0 = xin.tile([P, KT, TG, S], F16, tag="xt")
            for q in range(KT // 2):
                nc.gpsimd.dma_start(
                    out=xt0[:, 2 * q : 2 * q + 2],
                    in_=x_ap[0, :, 2 * q : 2 * q + 2],
                )

            gw_sb = const.tile([P, KT, F], F16)
            for q in range(KT // 2):
                nc.scalar.dma_start(
                    out=gw_sb[:, 2 * q : 2 * q + 2],
                    in_=gw_ap[:, 2 * q : 2 * q + 2],
                )
            # aw/bw queue BEHIND gw on Scalar: their 1MB is needed only
            # ~25us later, and the per-queue FIFO keeps them from
            # stealing packet slots from the critical gw/x0 stream.
            aw_sb = const.tile([P, KT, MT_A, P], F16)
            nc.scalar.dma_start(out=aw_sb[:], in_=aw_ap)
            bw_sb = const.tile([P, KT, MT_A, P], F16)
            nc.scalar.dma_start(out=bw_sb[:], in_=bw_ap)

            # Sync queue: the single packed-bias DMA (needed ~20us);
            # steady-state stores follow.
            bias_sb = const.tile([P, F + 2 * MT_A], F16)
            nc.sync.dma_start(out=bias_sb[:], in_=bias_ap)
            gbb_sb = bias_sb[:, :F]
            # tensor_scalar_add wants fp32 scalars: one-time upconvert
            abbb_sb = const.tile([P, 2 * MT_A], F32)
            nc.vector.tensor_copy(abbb_sb[:], bias_sb[:, F : F + 2 * MT_A])
            ab_sb = abbb_sb[:, :MT_A]
            bb_sb = abbb_sb[:, MT_A : 2 * MT_A]

            def emit_theta_phi(xt, half):
                """theta/phi for t-slice pair (2*half, 2*half+1): N=512."""
                thetaT = proj.tile([P, MT_A, 2, S], F16, tag="thetaT")
                phiT = proj.tile([P, MT_A, 2, S], F16, tag="phiT")
                pss = [
                    [
                        ps_a.tile([P, NF], F32, tag="ps", name=f"ps_p{pj}m{mt}")
                        for mt in range(MT_A)
                    ]
                    for pj in range(2)
                ]
                for kt in range(KT):
                    for pj, w_sb in enumerate((aw_sb, bw_sb)):
                        for mt in range(MT_A):
                            nc.tensor.matmul(
                                pss[pj][mt][:],
                                lhsT=w_sb[:, kt, mt, :],
                                rhs=xt[:, kt, 2 * half : 2 * half + 2, :],
                                start=(kt == 0),
                                stop=(kt == KT - 1),
                            )
                # theta evicts on Scalar (Identity: out = in + bias),
                # phi on Vector — the two chains run in parallel so the
                # ps_a slots free in half the time for the next stage.
                for mt in range(MT_A):
                    nc.scalar.activation(
                        out=thetaT[:, mt],
                        in_=pss[0][mt][:],
                        func=mybir.ActivationFunctionType.Identity,
                        bias=ab_sb[:, mt : mt + 1],
                    )
                    nc.vector.tensor_scalar_add(
                        phiT[:, mt], pss[1][mt][:], bb_sb[:, mt : mt + 1]
                    )
                return thetaT, phiT

            def emit_attn(thetaT, phiT, tip):
                """attnT[r, s] for one t; scale folded into the eviction,
                which is split Scalar/Vector to halve the chain."""
                attnT = asb.tile([P, MT_S, S], F16, tag="attnT")
                for rt in range(MT_S):
                    ps = ps_a.tile([P, NF], F32, tag="ps")
                    for at in range(MT_A):
                        nc.tensor.matmul(
                            ps[:, :S],
                            lhsT=phiT[:, at, tip, rt * P : (rt + 1) * P],
                            rhs=thetaT[:, at, tip, :],
                            start=(at == 0),
                            stop=(at == MT_A - 1),
                        )
                    if rt == 0:
                        nc.scalar.activation(
                            out=attnT[:, rt, :],
                            in_=ps[:, :S],
                            func=mybir.ActivationFunctionType.Copy,
                            scale=OUT_SCALE,
                        )
                    else:
                        nc.vector.tensor_scalar_mul(
                            attnT[:, rt, :], ps[:, :S], OUT_SCALE
                        )
                return attnT

            def emit_feats_pair_kt_outer(xt, ta, tb):
                """feats for a PAIR of t-slices jointly, kt-outer across
                all 8 PSUM banks: per arriving x kt-pair quarter there are
                2t x 2kt x 4 = 16 matmuls (~3.4us) — matching the ~3us
                quarter arrival cadence so the PE never starves during
                startup."""
                psf = {}
                for ti, pool in ((ta, ps_b), (tb, ps_a)):
                    for i in range(MT_S * NC_F):
                        psf[ti, i] = pool.tile(
                            [P, NF], F32, tag="ps", name=f"psf{ti}_{i}"
                        )
                for kt in range(KT):
                    for ti in (ta, tb):
                        for mt in range(MT_S):
                            for c in range(NC_F):
                                nc.tensor.matmul(
                                    psf[ti, mt * NC_F + c][:],
                                    lhsT=xt[:, kt, ti, mt * P : (mt + 1) * P],
                                    rhs=gw_sb[:, kt, c * NF : (c + 1) * NF],
                                    start=(kt == 0),
                                    stop=(kt == KT - 1),
                                )
                # tb's psums (ps_a) evict FIRST: the next stage allocates
                # from ps_a, so freeing those slots early keeps the PE fed.
                out = {}
                for ti in (tb, ta):
                    feats = fsb.tile([P, MT_S, F], F16, tag="feats")
                    for mt in range(MT_S):
                        for c in range(NC_F):
                            nc.vector.tensor_add(
                                feats[:, mt, c * NF : (c + 1) * NF],
                                psf[ti, mt * NC_F + c][:],
                                gbb_sb[:, c * NF : (c + 1) * NF],
                            )
                    out[ti] = feats
                return out[ta], out[tb]

            def emit_feats(xt, ti, pool):
                """feats[s, f] for one t (kt-inner; evictions interleave
                with the following psum group's matmuls)."""
                feats = fsb.tile([P, MT_S, F], F16, tag="feats")
                for mt in range(MT_S):
                    for c in range(NC_F):
                        ps = pool.tile([P, NF], F32, tag="ps", name="ps_f")
                        for kt in range(KT):
                            nc.tensor.matmul(
                                ps[:],
                                lhsT=xt[:, kt, ti, mt * P : (mt + 1) * P],
                                rhs=gw_sb[:, kt, c * NF : (c + 1) * NF],
                                start=(kt == 0),
                                stop=(kt == KT - 1),
                            )
                        nc.vector.tensor_add(
                            feats[:, mt, c * NF : (c + 1) * NF],
                            ps[:],
                            gbb_sb[:, c * NF : (c + 1) * NF],
                        )
                return feats

            def emit_out(t, attnT, feats, last=False):
                """out[s, f] = sum_r attnT[r, s] feats[r, f]; the two chunk
                evictions run on Vector and Scalar in parallel, then one
                fp16 store per (t, mt) with 2KB rows. For the final t the
                evictions split into 256-col chunks on both engines and
                the stores fan out in partition halves across the three
                DMA-capable queues, cutting the end-of-kernel tail."""
                for mt in range(MT_S):
                    out_sb = osb.tile([P, F], F16, tag="out_sb")
                    for c in range(NC_F):
                        ps = ps_b.tile([P, NF], F32, tag="ps", name="ps_o")
                        for rt in range(MT_S):
                            nc.tensor.matmul(
                                ps[:],
                                lhsT=attnT[:, rt, mt * P : (mt + 1) * P],
                                rhs=feats[:, rt, c * NF : (c + 1) * NF],
                                start=(rt == 0),
                                stop=(rt == MT_S - 1),
                            )
                        if c == 0:
                            nc.vector.tensor_copy(
                                out_sb[:, c * NF : (c + 1) * NF], ps[:]
                            )
                        else:
                            nc.scalar.activation(
                                out=out_sb[:, c * NF : (c + 1) * NF],
                                in_=ps[:],
                                func=mybir.ActivationFunctionType.Copy,
                            )
                    if last and mt == MT_S - 1:
                        # final store: halves on Scalar+GpSimd (idle by
                        # now) so the tail is ~1.3us, not 2.6.
                        HP = P // 2
                        nc.scalar.dma_start(
                            out=out_ap[t, mt * P : mt * P + HP, :],
                            in_=out_sb[:HP],
                        )
                        nc.gpsimd.dma_start(
                            out=out_ap[t, mt * P + HP : (mt + 1) * P, :],
                            in_=out_sb[HP:],
                        )
                    else:
                        nc.sync.dma_start(
                            out=out_ap[t, mt * P : (mt + 1) * P, :],
                            in_=out_sb[:],
                        )

            for g in range(NG):
                if g == 0:
                    xt = xt0
                    # Startup: feats for t0..t3 run as two joint kt-outer
                    # pairs, consuming x/gw quarters as they arrive;
                    # theta/phi start once a_w/b_w land behind gw.
                    f0sb, f1sb = emit_feats_pair_kt_outer(xt, 0, 1)
                    f2sb, f3sb = emit_feats_pair_kt_outer(xt, 2, 3)
                    th0, ph0 = emit_theta_phi(xt, 0)
                    th1, ph1 = emit_theta_phi(xt, 1)
                    a0 = emit_attn(th0, ph0, 0)
                    a1 = emit_attn(th0, ph0, 1)
                    emit_out(0, a0, f0sb)
                    a2 = emit_attn(th1, ph1, 0)
                    emit_out(1, a1, f1sb)
                    a3 = emit_attn(th1, ph1, 1)
                    emit_out(2, a2, f2sb)
                    emit_out(3, a3, f3sb)
                else:
                    xt = xin.tile([P, KT, TG, S], F16, tag="xt")
                    nc.gpsimd.dma_start(out=xt[:], in_=x_ap[g])
                    th0, ph0 = emit_theta_phi(xt, 0)
                    th1, ph1 = emit_theta_phi(xt, 1)
                    projs = [
                        (th0, ph0, 0), (th0, ph0, 1), (th1, ph1, 0), (th1, ph1, 1)
                    ]
                    for ti, (thp, php, tip) in enumerate(projs):
                        t = g * TG + ti
                        at_t = emit_attn(thp, php, tip)
                        f_t = emit_feats(xt, ti, ps_b)
                        emit_out(t, at_t, f_t, last=(t == T_LOC - 1))

    nc.compile()
    return nc


def _get_compiled():
    global _COMPILED
    if _COMPILED is None:
        _COMPILED = _build()
    return _COMPILED


def _prep_inputs(inputs):
    x = np.asarray(inputs["batch_data"], dtype=np.float32)
    assert x.shape == (T * S, F), x.shape
    # (T, S, F) -> per-core (T_LOC, F, S) -> (NG, TG, KT, P, S) -> (NG, P, KT, TG, S)
    x16 = (
        x.reshape(T, S, F)
        .transpose(0, 2, 1)
        .astype(np.float16)
        .reshape(N_CORES, NG, TG, KT, P, S)
        .transpose(0, 1, 4, 3, 2, 5)
    )
    x16 = np.ascontiguousarray(x16)

    def tile_w(w, mt):  # (F, N) -> (P, KT, mt, 128)
        n = w.shape[1]
        return np.ascontiguousarray(
            w.astype(np.float16).reshape(KT, P, mt, n // mt).transpose(1, 0, 2, 3)
        )

    aw16 = tile_w(np.asarray(inputs["a_w"], np.float32), MT_A)
    bw16 = tile_w(np.asarray(inputs["b_w"], np.float32), MT_A)
    gw16 = tile_w(np.asarray(inputs["g_w"], np.float32), 1).reshape(P, KT, F)
    # packed biases: [gb replicated | ab^T | bb^T] per partition, fp16
    ab = np.asarray(inputs["a_b"], np.float32).reshape(MT_A, P).T
    bb = np.asarray(inputs["b_b"], np.float32).reshape(MT_A, P).T
    gb = np.asarray(inputs["g_b"], np.float32)
    bias16 = np.ascontiguousarray(
        np.concatenate(
            [np.broadcast_to(gb, (P, F)), ab, bb], axis=1
        ).astype(np.float16)
    )
    in_maps = []
    for c in range(N_CORES):
        in_maps.append(
            {
                "x": x16[c],
                "aw": aw16,
                "bw": bw16,
                "gw": gw16,
                "bias": bias16,
            }
        )
    return in_maps


def run_spmd(inputs, **kwargs):
    """Run the compiled kernel; returns (full_output, BassKernelResults)."""
    nc = _get_compiled()
    in_maps = _prep_inputs(inputs)
    res = run_bass_kernel_spmd(nc, in_maps, list(range(N_CORES)), **kwargs)
    out = np.concatenate(
        [
            np.asarray(res.results[c]["out"]).astype(np.float32)
            for c in range(N_CORES)
        ],
        axis=0,
    )
    return out, res


def kernel(**inputs) -> np.ndarray:
    out, _ = run_spmd(inputs)
    return out



# revision 55
# speedup vs baseline: 1.0023x; 1.0023x over previous
"""Trainium2 Bass kernel for nn_CrossInferenceBlock (bilinear cross attention).

Computation (T=256, S=256, F=1024, A=256):
    theta = (x @ a_w + a_b).reshape(T, S, A)
    phi   = (x @ b_w + b_b).reshape(T, S, A)
    feats = (x @ g_w + g_b).reshape(T, S, F)
    attn  = einsum("tsa,tra->tsr", theta, phi)
    out   = einsum("tsr,trf->tsf", attn, feats) / (S + T)

Sharding: data-parallel over t — each of the 8 cores takes 32 contiguous
t-slices; the Linear weights are replicated.

The fp16 matmul stream (1792 N=512 + 128 N=256 MMs/core) is the
roofline: ~401us at the warm 2.4GHz issue rate (N/2.4 + 2.5ns NX).
Everything else is startup/tail engineering, driven by trace analysis:

  - The DMA subsystem is PACKET-RATE limited (~100 packets/us aggregate
    across queues; packet size = SBUF-side per-partition contiguous
    run). 2KB-run DMAs cap a queue near 50GB/s; 16KB runs reach the
    full ~358GB/s. Completion semaphores ride the same packet path, so
    a chunk's "ready" signal trails later chunks' data by ~1-3us.
  - Startup: group-0 x streams on GpSimd as four kt-pair chunks (4KB
    runs — chunk k lands ~gate+1.4k us, ahead of the ~gate+1.7k us
    consumption front); gw mirrors that on Scalar. aw/bw (needed only
    ~25us in) queue BEHIND gw on Scalar so the per-queue FIFO keeps
    them out of the critical window. All small biases are host-packed
    into ONE [P, F+4] fp16 tensor (a transposed ab/bb load would emit
    256 tiny packets that clog the packet-limited phase).
  - ~150 dummy warm-up matmuls bridge engine-init (~7.5us) to first
    data (~17us): PE_HAM reaches 8/8 (2.4GHz) at ~11us and the PE is
    never idle after, so the clock gate never re-throttles (one idle
    MID window would cost ~10us).
  - Group 0 computes feats for t-PAIRS jointly, kt-outer, across all 8
    PSUM banks: 16 MMs (~3.4us) per arriving x/gw kt chunk matches the
    delivery cadence. theta/phi follow when a_w/b_w land; ps_a-pool
    psums evict first so the next stage's banks free earliest.
  - theta/phi matmuls process a PAIR of t-slices per instruction (N=512).
  - PSUM evictions alternate Vector/Scalar so eviction chains never gate
    the PE; output is stored fp16 (host upcasts), 2KB DMA rows, one
    store per (t, mt) on Sync; the final store splits across the
    Scalar/GpSimd queues to shorten the end-of-kernel tail.

Matmuls run in fp16 (fp32 PSUM accumulation). fp8 was evaluated and
rejected: TRN e4m3 gives ~3.5% rel error on this data (tolerance 2e-2)
and DoubleRow is only ~1.5x, so no precision/speed tradeoff works.
N=1024 matmuls (2-bank PSUM) are rejected by the backend ISA check.
Note: under sustained load the chip can enter P0 (PE ~2.0GHz), which
inflates measurements by ~1.2x — compare runs on a cool device.
"""

import numpy as np

import concourse.bass as bass
import concourse.bacc as bacc
import concourse.tile as tile
from concourse import mybir
from concourse.bass_utils import run_bass_kernel_spmd

T, S, F, A = 256, 256, 1024, 256
N_CORES = 8
T_LOC = T // N_CORES          # 32 t-slices per core
P = 128
KT = F // P                   # 8 contraction tiles over F
MT_A = A // P                 # 2 output tiles over A
MT_S = S // P                 # 2 tiles over s (rows of one t-slice)
NF = 512                      # matmul free-dim chunk for F-wide outputs
NC_F = F // NF                # 2 chunks
TG = 4                        # t-slices fetched per input DMA group
NG = T_LOC // TG              # 8 DMA groups per core
N_WARM = 150                  # warm-up matmuls bridging until data lands (~18.5us)
OUT_SCALE = 1.0 / (S + T)

F16 = mybir.dt.float16
F32 = mybir.dt.float32

_COMPILED = None


def _build():
    nc = bacc.Bacc("TRN2", target_bir_lowering=False, debug=False)

    # All inputs are host-prearranged so every DMA reads per-partition
    # CONTIGUOUS runs, keeping HBM transfers at full rate.
    # x: (NG, P, KT, TG, S) with t = g*TG + ti, f = kt*P + p.
    x_d = nc.dram_tensor("x", [NG, P, KT, TG, S], F16, kind="ExternalInput")
    aw_d = nc.dram_tensor("aw", [P, KT, MT_A, P], F16, kind="ExternalInput")
    bw_d = nc.dram_tensor("bw", [P, KT, MT_A, P], F16, kind="ExternalInput")
    gw_d = nc.dram_tensor("gw", [P, KT, F], F16, kind="ExternalInput")
    # biases: host-packed per-partition rows [gb_rep(F) | ab(MT_A) | bb(MT_A)]
    # fp16 — ONE 128-packet DMA instead of a broadcast + two transposed
    # tiny-packet DMAs that would clog the packet-limited startup phase.
    bias_d = nc.dram_tensor(
        "bias", [P, F + 2 * MT_A], F16, kind="ExternalInput"
    )
    out_d = nc.dram_tensor("out", [T_LOC, S, F], F16, kind="ExternalOutput")

    x_ap = x_d.ap()
    aw_ap = aw_d.ap()
    bw_ap = bw_d.ap()
    gw_ap = gw_d.ap()
    bias_ap = bias_d.ap()
    out_ap = out_d.ap()

    with tile.TileContext(nc) as tc:
        with (
            tc.tile_pool(name="const", bufs=1) as const,
            tc.tile_pool(name="xin", bufs=3) as xin,
            tc.tile_pool(name="proj", bufs=3) as proj,
            tc.tile_pool(name="fsb", bufs=6) as fsb,
            tc.tile_pool(name="asb", bufs=4) as asb,
            tc.tile_pool(name="osb", bufs=6) as osb,
            tc.tile_pool(name="ps_a", bufs=4, space="PSUM") as ps_a,
            tc.tile_pool(name="ps_b", bufs=4, space="PSUM") as ps_b,
        ):
            # --- PE_HAM warm-up: dummy matmuls on a memset tile. These
            # depend only on engine init (~6.5us), not on any DMA, so the
            # clock gate reaches 8/8 before the first real matmul.
            warm_sb = const.tile([P, P], F16)
            nc.vector.memset(warm_sb[:], 0.0)
            warm_ps = ps_a.tile([P, NF], F32, tag="ps")
            for _ in range(N_WARM):
                nc.tensor.matmul(
                    warm_ps[:, :P], lhsT=warm_sb[:], rhs=warm_sb[:],
                    start=True, stop=True,
                )

            # --- Startup DMAs. The DMA subsystem is packet-rate limited
            # (~100 packets/us aggregate; packet size = SBUF-side
            # per-partition run). Group-0 x streams as a kt-QUAD (8KB
            # runs) then two kt-pairs: the quad has the same packet count
            # as a pair (so the first-matmul gate is no later) but lands
            # 2x the work backlog, absorbing DMA jitter. gw mirrors this
            # with a half + two pairs on Scalar.
            # Four kt-PAIR chunks (4KB runs) for both streams. Measured
            # against alternatives: kt-singles gate ~1us earlier but
            # double the early packet count and starve mid-startup; a
            # coarse kt4-7 quad halves packets but its data+semaphore
            # loses the race against the f01 consumption front (2us PE
            # gap on slow-DMA runs). Pairs starve nowhere: chunk k lands
            # at ~gate+1.4k us vs the ~gate+1.7k us consumption front.
            xt0 = xin.tile([P, KT, TG, S], F16, tag="xt")
            for q in range(KT // 2):
                nc.gpsimd.dma_start(
                    out=xt0[:, 2 * q : 2 * q + 2],
                    in_=x_ap[0, :, 2 * q : 2 * q + 2],
                )

            gw_sb = const.tile([P, KT, F], F16)
            for q in range(KT // 2):
                nc.scalar.dma_start(
                    out=gw_sb[:, 2 * q : 2 * q + 2],
                    in_=gw_ap[:, 2 * q : 2 * q + 2],
                )
            # aw/bw queue BEHIND gw on Scalar: their 1MB is needed only
            # ~25us later, and the per-queue FIFO keeps them from
            # stealing packet slots from the critical gw/x0 stream.
            aw_sb = const.tile([P, KT, MT_A, P], F16)
            nc.scalar.dma_start(out=aw_sb[:], in_=aw_ap)
            bw_sb = const.tile([P, KT, MT_A, P], F16)
            nc.scalar.dma_start(out=bw_sb[:], in_=bw_ap)

            # Sync queue: the single packed-bias DMA (needed ~20us);
            # steady-state stores follow.
            bias_sb = const.tile([P, F + 2 * MT_A], F16)
            nc.sync.dma_start(out=bias_sb[:], in_=bias_ap)
            gbb_sb = bias_sb[:, :F]
            # tensor_scalar_add wants fp32 scalars: one-time upconvert
            abbb_sb = const.tile([P, 2 * MT_A], F32)
            nc.vector.tensor_copy(abbb_sb[:], bias_sb[:, F : F + 2 * MT_A])
            ab_sb = abbb_sb[:, :MT_A]
            bb_sb = abbb_sb[:, MT_A : 2 * MT_A]

            def emit_theta_phi(xt, half):
                """theta/phi for t-slice pair (2*half, 2*half+1): N=512."""
                thetaT = proj.tile([P, MT_A, 2, S], F16, tag="thetaT")
                phiT = proj.tile([P, MT_A, 2, S], F16, tag="phiT")
                pss = [
                    [
                        ps_a.tile([P, NF], F32, tag="ps", name=f"ps_p{pj}m{mt}")
                        for mt in range(MT_A)
                    ]
                    for pj in range(2)
                ]
                for kt in range(KT):
                    for pj, w_sb in enumerate((aw_sb, bw_sb)):
                        for mt in range(MT_A):
                            nc.tensor.matmul(
                                pss[pj][mt][:],
                                lhsT=w_sb[:, kt, mt, :],
                                rhs=xt[:, kt, 2 * half : 2 * half + 2, :],
                                start=(kt == 0),
                                stop=(kt == KT - 1),
                            )
                # theta evicts on Scalar (Identity: out = in + bias),
                # phi on Vector — the two chains run in parallel so the
                # ps_a slots free in half the time for the next stage.
                for mt in range(MT_A):
                    nc.scalar.activation(
                        out=thetaT[:, mt],
                        in_=pss[0][mt][:],
                        func=mybir.ActivationFunctionType.Identity,
                        bias=ab_sb[:, mt : mt + 1],
                    )
                    nc.vector.tensor_scalar_add(
                        phiT[:, mt], pss[1][mt][:], bb_sb[:, mt : mt + 1]
                    )
                return thetaT, phiT

            def emit_attn(thetaT, phiT, tip):
                """attnT[r, s] for one t; scale folded into the eviction,
                which is split Scalar/Vector to halve the chain."""
                attnT = asb.tile([P, MT_S, S], F16, tag="attnT")
                for rt in range(MT_S):
                    ps = ps_a.tile([P, NF], F32, tag="ps")
                    for at in range(MT_A):
                        nc.tensor.matmul(
                            ps[:, :S],
                            lhsT=phiT[:, at, tip, rt * P : (rt + 1) * P],
                            rhs=thetaT[:, at, tip, :],
                            start=(at == 0),
                            stop=(at == MT_A - 1),
                        )
                    if rt == 0:
                        nc.scalar.activation(
                            out=attnT[:, rt, :],
                            in_=ps[:, :S],
                            func=mybir.ActivationFunctionType.Copy,
                            scale=OUT_SCALE,
                        )
                    else:
                        nc.vector.tensor_scalar_mul(
                            attnT[:, rt, :], ps[:, :S], OUT_SCALE
                        )
                return attnT

            def emit_feats_pair_kt_outer(xt, ta, tb):
                """feats for a PAIR of t-slices jointly, kt-outer across
                all 8 PSUM banks: per arriving x kt-pair quarter there are
                2t x 2kt x 4 = 16 matmuls (~3.4us) — matching the ~3us
                quarter arrival cadence so the PE never starves during
                startup."""
                psf = {}
                for ti, pool in ((ta, ps_b), (tb, ps_a)):
                    for i in range(MT_S * NC_F):
                        psf[ti, i] = pool.tile(
                            [P, NF], F32, tag="ps", name=f"psf{ti}_{i}"
                        )
                for kt in range(KT):
                    for ti in (ta, tb):
                        for mt in range(MT_S):
                            for c in range(NC_F):
                                nc.tensor.matmul(
                                    psf[ti, mt * NC_F + c][:],
                                    lhsT=xt[:, kt, ti, mt * P : (mt + 1) * P],
                                    rhs=gw_sb[:, kt, c * NF : (c + 1) * NF],
                                    start=(kt == 0),
                                    stop=(kt == KT - 1),
                                )
                # tb's psums (ps_a) evict FIRST: the next stage allocates
                # from ps_a, so freeing those slots early keeps the PE fed.
                out = {}
                for ti in (tb, ta):
                    feats = fsb.tile([P, MT_S, F], F16, tag="feats")
                    for mt in range(MT_S):
                        for c in range(NC_F):
                            nc.vector.tensor_add(
                                feats[:, mt, c * NF : (c + 1) * NF],
                                psf[ti, mt * NC_F + c][:],
                                gbb_sb[:, c * NF : (c + 1) * NF],
                            )
                    out[ti] = feats
                return out[ta], out[tb]

            def emit_feats(xt, ti, pool):
                """feats[s, f] for one t (kt-inner; evictions interleave
                with the following psum group's matmuls)."""
                feats = fsb.tile([P, MT_S, F], F16, tag="feats")
                for mt in range(MT_S):
                    for c in range(NC_F):
                        ps = pool.tile([P, NF], F32, tag="ps", name="ps_f")
                        for kt in range(KT):
                            nc.tensor.matmul(
                                ps[:],
                                lhsT=xt[:, kt, ti, mt * P : (mt + 1) * P],
                                rhs=gw_sb[:, kt, c * NF : (c + 1) * NF],
                                start=(kt == 0),
                                stop=(kt == KT - 1),
                            )
                        nc.vector.tensor_add(
                            feats[:, mt, c * NF : (c + 1) * NF],
                            ps[:],
                            gbb_sb[:, c * NF : (c + 1) * NF],
                        )
                return feats

            def emit_out(t, attnT, feats, last=False):
                """out[s, f] = sum_r attnT[r, s] feats[r, f]; the two chunk
                evictions run on Vector and Scalar in parallel, then one
                fp16 store per (t, mt) with 2KB rows. For the final t the
                evictions split into 256-col chunks on both engines and
                the stores fan out in partition halves across the three
                DMA-capable queues, cutting the end-of-kernel tail."""
                for mt in range(MT_S):
                    out_sb = osb.tile([P, F], F16, tag="out_sb")
                    for c in range(NC_F):
                        ps = ps_b.tile([P, NF], F32, tag="ps", name="ps_o")
                        for rt in range(MT_S):
                            nc.tensor.matmul(
                                ps[:],
                                lhsT=attnT[:, rt, mt * P : (mt + 1) * P],
                                rhs=feats[:, rt, c * NF : (c + 1) * NF],
                                start=(rt == 0),
                                stop=(rt == MT_S - 1),
                            )
                        if c == 0:
                            nc.vector.tensor_copy(
                                out_sb[:, c * NF : (c + 1) * NF], ps[:]
                            )
                        else:
                            nc.scalar.activation(
                                out=out_sb[:, c * NF : (c + 1) * NF],
                                in_=ps[:],
                                func=mybir.ActivationFunctionType.Copy,
                            )
                    if last and mt == MT_S - 1:
                        # final store: halves on Scalar+GpSimd (idle by
                        # now) so the tail is ~1.3us, not 2.6.
                        HP = P // 2
                        nc.scalar.dma_start(
                            out=out_ap[t, mt * P : mt * P + HP, :],
                            in_=out_sb[:HP],
                        )
                        nc.gpsimd.dma_start(
                            out=out_ap[t, mt * P + HP : (mt + 1) * P, :],
                            in_=out_sb[HP:],
                        )
                    else:
                        nc.sync.dma_start(
                            out=out_ap[t, mt * P : (mt + 1) * P, :],
                            in_=out_sb[:],
                        )

            for g in range(NG):
                if g == 0:
                    xt = xt0
                    # Startup: feats for t0..t3 run as two joint kt-outer
                    # pairs, consuming x/gw quarters as they arrive;
                    # theta/phi start once a_w/b_w land behind gw.
                    f0sb, f1sb = emit_feats_pair_kt_outer(xt, 0, 1)
                    f2sb, f3sb = emit_feats_pair_kt_outer(xt, 2, 3)
                    th0, ph0 = emit_theta_phi(xt, 0)
                    th1, ph1 = emit_theta_phi(xt, 1)
                    a0 = emit_attn(th0, ph0, 0)
                    a1 = emit_attn(th0, ph0, 1)
                    emit_out(0, a0, f0sb)
                    a2 = emit_attn(th1, ph1, 0)
                    emit_out(1, a1, f1sb)
                    a3 = emit_attn(th1, ph1, 1)
                    emit_out(2, a2, f2sb)
                    emit_out(3, a3, f3sb)
                else:
                    xt = xin.tile([P, KT, TG, S], F16, tag="xt")
                    nc.gpsimd.dma_start(out=xt[:], in_=x_ap[g])
                    th0, ph0 = emit_theta_phi(xt, 0)
                    th1, ph1 = emit_theta_phi(xt, 1)
                    projs = [
                        (th0, ph0, 0), (th0, ph0, 1), (th1, ph1, 0), (th1, ph1, 1)
                    ]
                    for ti, (thp, php, tip) in enumerate(projs):
                        t = g * TG + ti
                        at_t = emit_attn(thp, php, tip)
                        f_t = emit_feats(xt, ti, ps_b)
                        emit_out(t, at_t, f_t, last=(t == T_LOC - 1))

    nc.compile()
    return nc


def _get_compiled():
    global _COMPILED
    if _COMPILED is None:
        _COMPILED = _build()
    return _COMPILED


def _prep_inputs(inputs):
    x = np.asarray(inputs["batch_data"], dtype=np.float32)
    assert x.shape == (T * S, F), x.shape
    # (T, S, F) -> per-core (T_LOC, F, S) -> (NG, TG, KT, P, S) -> (NG, P, KT, TG, S)
    x16 = (
        x.reshape(T, S, F)
        .transpose(0, 2, 1)
        .astype(np.float16)
        .reshape(N_CORES, NG, TG, KT, P, S)
        .transpose(0, 1, 4, 3, 2, 5)
    )
    x16 = np.ascontiguousarray(x16)

    def tile_w(w, mt):  # (F, N) -> (P, KT, mt, 128)
        n = w.shape[1]
        return np.ascontiguousarray(
            w.astype(np.float16).reshape(KT, P, mt, n // mt).transpose(1, 0, 2, 3)
        )

    aw16 = tile_w(np.asarray(inputs["a_w"], np.float32), MT_A)
    bw16 = tile_w(np.asarray(inputs["b_w"], np.float32), MT_A)
    gw16 = tile_w(np.asarray(inputs["g_w"], np.float32), 1).reshape(P, KT, F)
    # packed biases: [gb replicated | ab^T | bb^T] per partition, fp16
    ab = np.asarray(inputs["a_b"], np.float32).reshape(MT_A, P).T
    bb = np.asarray(inputs["b_b"], np.float32).reshape(MT_A, P).T
    gb = np.asarray(inputs["g_b"], np.float32)
    bias16 = np.ascontiguousarray(
        np.concatenate(
            [np.broadcast_to(gb, (P, F)), ab, bb], axis=1
        ).astype(np.float16)
    )
    in_maps = []
    for c in range(N_CORES):
        in_maps.append(
            {
                "x": x16[c],
                "aw": aw16,
                "bw": bw16,
                "gw": gw16,
                "bias": bias16,
            }
        )
    return in_maps


def run_spmd(inputs, **kwargs):
    """Run the compiled kernel; returns (full_output, BassKernelResults)."""
    nc = _get_compiled()
    in_maps = _prep_inputs(inputs)
    res = run_bass_kernel_spmd(nc, in_maps, list(range(N_CORES)), **kwargs)
    out = np.concatenate(
        [
            np.asarray(res.results[c]["out"]).astype(np.float32)
            for c in range(N_CORES)
        ],
        axis=0,
    )
    return out, res


def kernel(**inputs) -> np.ndarray:
    out, _ = run_spmd(inputs)
    return out

